# revision 39
# baseline (speedup 1.0000x reference)
"""Trainium2 Bass kernel for nn_MgSmmSModel_85220741088115 (self-contained).

The reference model is a linear RNN over T=512 steps whose output is a single
scalar per batch element:
  h_t = x_proj_t + h_{t-1} @ W_hc.T;  out = (hT @ W_h.T + ...) @ W_1d.T + b_1d
Because the readout is rank-1, the whole model collapses to a batch-independent
weight functional plus a short dot product over the last J timesteps:
  out[b] = sum_j alpha_j x[b,T-1-j] + s_x x[b,T-1] + beta + c0
  alpha_j = w1d . (W_h W_hc^j w_ic) = u0 . v_j   (u0 = W_h^T w1d, v_j = W_hc^j w_ic)
  beta    = sum_j u0 . y_j                        (y_j = W_hc^j (b_ic+b_hc+b_c))
  c0 = w1d . (b_h + b_g + b_x + rowsum(W_g)) + b_1d;  s_x = w1d . W_x[:,0]
The chain contracts at rho(W_hc) ~ 0.59/step. Weights are fp16 except W_h
(fp8 e4m3 -- u0 tolerates it). J=9 measures 7.122e-3 rel error on hardware,
matching the fp64 host model of the same quantization (2.8x under the
2e-2 gate).

Schedule (per core; all 8 cores run the same program on a batch shard):
  - W_hc^T and W_g^T load via HWDGE (SP queue, whct first); W_h (fp8)
    loads via a *prepared* SWDGE gather (PREPARE_ONLY descriptor gen on
    Pool, fired by trigger_dma) whose completion-to-consumer latency is
    much shorter than the HWDGE path. One prepared load is the sweet spot:
    two or more serialize against each other and regress. The gather idx
    tile holds the wrapped [16,8] iota replicated to all 8 groups of 16
    partitions (HW-verified layout; the consumer uses explicit wait_ge on
    the descriptor DMA semaphore since the prepared-gather dst RAW edge is
    not tile-managed).
  - c0/s_x are computed as a pure bilinear matmul (lhsT chunk k =
    [wx_k | (bh+bg+bx+rowsum)_k] vs rhs w1d_k, plus a b1d column vs ones)
    so only one DVE op sits between W_g^T arriving and the epilogue.
  - The *forward* v/y chain needs only W_hc, so it runs entirely under the
    W_g/W_h loads. Chain steps are column-layout: out[m-chunk][128,2]
    accumulated over 8 contract chunks -> 64 matmuls with out free size 2.
  - Seeds are scaled by 2^10 on device (fp16 subnormal guard); the alpha/beta
    column is scaled back by 2^-10 before the epilogue.
  - beta, s_x and c0 are all folded into the single epilogue matmul: the
    alpha column interleaves (alpha_j, beta_j) in rows 0..23, s_x sits at
    row 32 and c0+b_1d at row 33 (PSUM col base must be 0/32/64/96), and
    xt2 holds x values / 1.0 / x[T-1] / 1.0 in the matching rows.
  - The output store is also a prepared SWDGE scatter-add into a pre-zeroed
    64-f32 DRAM row, fired by a trigger after the final copy -- this skips
    the ~1.3us HWDGE+DGE latency a plain output dma_start would pay.

SPMD over 8 NeuronCores: weight work is replicated (no cross-core collectives:
they cost ~15us flat in the cost model); the batch dim (128) is sharded 16 per
core for the epilogue. Host code does layout/sharding/dtype-cast only.
"""

import numpy as np
import sys
sys.path.insert(0, '/opt/trn_rl_repo')
from concourse import bass, bacc, tile, mybir

F32 = mybir.dt.float32
F16 = mybir.dt.float16
F8 = mybir.dt.float8e4
WH_NP = 'float8_e4m3'   # W_h dtype: u0 tolerates e4m3 (7.2e-3 host rel err)

H = 1024
KT = 8          # 1024 / 128 partition chunks
GT = 4          # 512 / 128 partition chunks (W_g^T rows)
T = 512
B = 128
N_CORES = 8
J = 9           # chain length
C2 = 2 * J      # interleaved (alpha, beta) rows
AB = 34         # ab column length: [0:24] interleaved, 32 = s_x, 33 = c0+b1d
B_SH = B // N_CORES
SC_UP = 1024.0
SC_DN = 1.0 / 1024.0


def col_layout(vec):
    """[1024] -> [128, 8] with element (p, k) = vec[k*128 + p]."""
    return np.ascontiguousarray(vec.reshape(KT, 128).T).astype(np.float32)


def pmaj(mat, nchunks):
    """[nchunks*128, H] -> [128, nchunks*H] with (p, k*H+f) = mat[k*128+p, f]."""
    return np.ascontiguousarray(
        mat.reshape(nchunks, 128, H).transpose(1, 0, 2).reshape(128, nchunks * H))


def _gidx():
    """Gather index tile: wrapped iota (idx[p%16, s] = s*16+p%16), replicated
    to all 8 groups of 16 partitions (one per SDMA engine -- HW-verified)."""
    g = np.zeros((128, KT), np.int16)
    blk = np.arange(128, dtype=np.int16).reshape(KT, 16).T
    for grp in range(8):
        g[grp * 16:(grp + 1) * 16, :] = blk
    return g


def prep_inputs(inputs):
    """Host-side layout/dtype prep only (no arithmetic). -> (replicated, per_core)."""
    x = inputs['x']
    rep = {
        'whct': pmaj(np.ascontiguousarray(inputs['W_hc'].T), KT).astype(np.float16),
        'wh': pmaj(np.asarray(inputs['W_h']), KT).astype(
            np.float16 if WH_NP == 'float16' else __import__('ml_dtypes').float8_e4m3),
        'wgt': pmaj(np.ascontiguousarray(inputs['W_g'].T), GT).astype(np.float16),
        'cols': np.concatenate([
            col_layout(inputs['W_1d'][0]),
            col_layout(inputs['W_ic'][:, 0]),
            col_layout(inputs['W_x'][:, 0]),
            col_layout(inputs['b_ic']),
            col_layout(inputs['b_hc']),
            col_layout(inputs['b_c']),
            col_layout(inputs['b_h']),
            col_layout(inputs['b_g']),
            col_layout(inputs['b_x'])], axis=1),
        'b1d': np.asarray(inputs['b_1d'], np.float32).reshape(1, 1),
        'gidx': _gidx(),
    }
    per_core = []
    for i in range(N_CORES):
        xs = x[i * B_SH:(i + 1) * B_SH, T - J:T, 0]      # [B_SH, J]
        xt2 = np.zeros((AB, B_SH), np.float32)
        xt2[0:C2:2, :] = np.ascontiguousarray(xs[:, ::-1].T)  # row 2j = x[., T-1-j]
        xt2[1:C2:2, :] = 1.0                                   # beta rows
        xt2[32, :] = x[i * B_SH:(i + 1) * B_SH, T - 1, 0]      # s_x row
        xt2[33, :] = 1.0                                        # c0 row
        per_core.append({'xt2': xt2})
    return rep, per_core


def build():
    nc = bacc.Bacc("TRN2", target_bir_lowering=False, debug=False,
                   num_devices=N_CORES, num_swdge_queues=3)

    dram = {}
    def din(name, shape, dt=F32):
        dram[name] = nc.dram_tensor(name, list(shape), dt, kind="ExternalInput").ap()
    din('whct', (128, KT * H), F16)
    din('wh', (128, KT * H), F16 if WH_NP == 'float16' else F8)
    din('wgt', (128, GT * H), F16)
    din('cols', (128, 9 * KT))
    din('b1d', (1, 1))
    din('xt2', (AB, B_SH))
    din('gidx', (128, KT), mybir.dt.int16)
    # out row padded to 64 f32 (scatter elem_size granularity is 256B);
    # host slices [:B_SH].
    out_d = nc.dram_tensor("out", [1, 64], F32, kind="ExternalOutput").ap()

    with tile.TileContext(nc) as tc:
        with (
            tc.tile_pool(name="const", bufs=1) as cpool,
            tc.tile_pool(name="psum", bufs=2, space="PSUM") as ppool,
            tc.tile_pool(name="psA", bufs=1, space="PSUM") as ppA,
            tc.tile_pool(name="psB", bufs=1, space="PSUM") as ppB,
            tc.tile_pool(name="psC", bufs=1, space="PSUM") as ppC,
        ):
            # ---- persistent SBUF tiles
            whct_sb = cpool.tile([128, 1, KT * H], F16, tag="whct")
            wh_sb = cpool.tile([128, 1, KT * H],
                               F16 if WH_NP == 'float16' else F8, tag="wh")
            wgt_sb = cpool.tile([128, 1, GT * H], F16, tag="wgt")
            cols_sb = cpool.tile([128, 9 * KT], F32, tag="cols")
            COL_ORDER = ('w1d_c', 'wic_c', 'wx_c', 'bic_c', 'bhc_c', 'bc_c',
                         'bh_c', 'bg_c', 'bx_c')
            colv = {n: cols_sb[:, i * KT:(i + 1) * KT]
                    for i, n in enumerate(COL_ORDER)}
            b1d_sb = cpool.tile([1, 1], F32, tag="b1d")
            xt2_sb = cpool.tile([AB, B_SH], F32, tag="xt2")
            VY = cpool.tile([128, KT, C2], F16, tag="VY")
            w1d16 = cpool.tile([128, KT], F16, tag="w1d16")
            u016 = cpool.tile([128, KT], F16, tag="u016")
            ones16 = cpool.tile([128, 1], F16, tag="ones16")
            onesf = cpool.tile([128, 1], F32, tag="onesf")
            seedf = cpool.tile([128, 2 * KT], F32, tag="seedf")
            bsum = cpool.tile([128, KT], F32, tag="bsum")
            bsum2 = cpool.tile([128, KT], F32, tag="bsum2")
            q2b = cpool.tile([128, KT, 2], F32, tag="q2b")
            b1dcol = cpool.tile([128, 2], F32, tag="b1dcol")
            ab_col = cpool.tile([AB, 1], F32, tag="ab_col")
            out_sb = cpool.tile([128, 1, 64], F32, tag="out_sb")
            oidx = cpool.tile([128, 1], mybir.dt.int16, tag="oidx")
            gidx_sb = cpool.tile([128, KT], mybir.dt.int16, tag="gidx")

            # ---- Transfers. whct first on HWDGE (SP); wgt/wh are prepared
            # SWDGE gathers whose triggers enqueue for the DMA engines in
            # Pool program order -> transfer order whct, (smalls), wgt, wh.
            # Triggered SWDGE completion skips the HWDGE post-transfer
            # latency, shortening the critical wh->u0 hop.
            nc.sync.dma_start(xt2_sb[:], dram['xt2'][:])
            nc.sync.dma_start(wgt_sb[:, 0, :], dram['wgt'][:])
            nc.sync.dma_start(wh_sb[:, 0, :], dram['wh'][:])
            nc.scalar.dma_start(gidx_sb[:], dram['gidx'][:])
            nc.scalar.dma_start(cols_sb[:], dram['cols'][:])
            nc.scalar.dma_start(b1d_sb[:], dram['b1d'][:])

            nc.vector.memset(onesf[:], 1.0)
            nc.vector.tensor_copy(ones16[:], onesf[:])
            nc.vector.memset(ab_col[:], 0.0)   # zeros dead rows 24..31
            nc.vector.memset(out_sb[:], 0.0)
            nc.gpsimd.memset(oidx[:], 0)
            # Pre-zero the out row (the scatter-add below accumulates into it)
            nc.scalar.dma_start(out_d[:], out_sb[0:1, 0, :])

            whct_dma_sem = nc.alloc_semaphore("whct_swdge_dma")
            out_dma_sem = nc.alloc_semaphore("out_swdge_dma")
            nc.gpsimd.dma_gather(
                whct_sb[:], dram['whct'][:], gidx_sb[:],
                128, 128, KT * H, prepare_only=True, sem=whct_dma_sem,
                queue_num=1)
            nc.gpsimd.trigger_dma(count=None, queue_num=1)
            # Output-store prep (fired by the trigger at the very end)
            nc.gpsimd.dma_scatter_add(
                out_d[:], out_sb[:], oidx[:], 1, 1, 64,
                prepare_only=True, sem=out_dma_sem, queue_num=2)

            # ---- seeds: v_0 = 2^10 * w_ic, y_0 = 2^10 * (b_ic+b_hc+b_c)
            nc.vector.tensor_scalar_mul(seedf[:, 0:KT], colv['wic_c'], SC_UP)
            nc.vector.tensor_add(bsum[:], colv['bic_c'], colv['bhc_c'])
            nc.vector.tensor_add(bsum[:], bsum[:], colv['bc_c'])
            nc.vector.tensor_scalar_mul(seedf[:, KT:2 * KT], bsum[:], SC_UP)
            nc.vector.tensor_copy(VY[:, :, 0], seedf[:, 0:KT])
            nc.vector.tensor_copy(VY[:, :, 1], seedf[:, KT:2 * KT])
            nc.vector.tensor_copy(w1d16[:], colv['w1d_c'])
            # early constants prep (Activation engine; all inputs are cols)
            nc.vector.tensor_add(bsum2[:], colv['bh_c'], colv['bg_c'])
            nc.vector.tensor_add(bsum2[:], bsum2[:], colv['bx_c'])
            nc.vector.tensor_copy(q2b[:, :, 0], colv['wx_c'])
            nc.vector.memset(b1dcol[:], 0.0)
            nc.vector.tensor_copy(b1dcol[0:1, 1:2], b1d_sb[:])

            # ---- chain: (v,y)_{j+1} = W_hc (v,y)_j, column layout.
            # lhsT tile (k,m) = W_hc^T[k-chunk, m-chunk]; rhs = VY[:, k, 2j:2j+2].
            nc.tensor.wait_ge(whct_dma_sem, 16)
            for j in range(J - 1):
                cp = ppool.tile([128, KT, 2], F32, tag="cp")
                for m in range(KT):
                    for k in range(KT):
                        nc.tensor.matmul(
                            cp[:, m, :],
                            whct_sb[:, 0, k * H + m * 128:k * H + (m + 1) * 128],
                            VY[:, k, 2 * j:2 * j + 2],
                            start=(k == 0), stop=(k == KT - 1))
                nc.vector.tensor_copy(VY[:, :, 2 * (j + 1):2 * (j + 1) + 2], cp[:])

            # ---- rowsum(W_g) via ones: rs[m-chunk] = sum_g W_g[m-chunk, g]
            rs = ppA.tile([128, KT], F32, tag="rs")
            for m in range(KT):
                for g in range(GT):
                    nc.tensor.matmul(
                        rs[:, m:m + 1],
                        wgt_sb[:, 0, g * H + m * 128:g * H + (m + 1) * 128],
                        ones16[:], start=(g == 0), stop=(g == GT - 1))

            # ---- s_x / c0 rows via a bilinear matmul at PSUM base 32:
            # abp[32] = sum wx.w1d, abp[33] = sum (bh+bg+bx+rowsum).w1d + b1d
            # (lhsT chunk k = [wx_k | (bsum2+rs)_k], rhs = w1d_k; the final
            # matmul adds b1d via a [128,2] column against ones).
            abp = ppC.tile([AB, 1], F32, tag="abp")
            nc.vector.tensor_add(q2b[:, :, 1], bsum2[:], rs[:])

            # ---- u0 = W_h^T w1d in column layout
            up = ppA.tile([128, KT], F32, tag="up")
            for m in range(KT):
                for k in range(KT):
                    nc.tensor.matmul(
                        up[:, m:m + 1],
                        wh_sb[:, 0, k * H + m * 128:k * H + (m + 1) * 128],
                        w1d16[:, k:k + 1],
                        start=(k == 0), stop=(k == KT - 1))
            nc.vector.tensor_copy(u016[:], up[:])
            # const matmuls (PE) after u0 so u0 isn't stalled behind them
            for k in range(KT):
                nc.tensor.matmul(abp[32:34, :], q2b[:, k, :],
                                 colv['w1d_c'][:, k:k + 1],
                                 start=(k == 0), stop=False)
            nc.tensor.matmul(abp[32:34, :], b1dcol[:], onesf[:],
                             start=False, stop=True)
            nc.vector.tensor_copy(ab_col[32:34, :], abp[32:34, :])

            # ---- interleaved (alpha_j, beta_j) rows = VY^T u0, scaled 2^-10
            for k in range(KT):
                nc.tensor.matmul(abp[0:C2, :], VY[:, k, :], u016[:, k:k + 1],
                                 start=(k == 0), stop=(k == KT - 1))
            nc.vector.tensor_scalar_mul(ab_col[0:C2, :], abp[0:C2, :], SC_DN)

            # ---- epilogue: out[1, B_SH] = ab_col^T @ xt2 (+ everything folded)
            op = ppB.tile([1, B_SH], F32, tag="op")
            nc.tensor.matmul(op[:], ab_col[:], xt2_sb[:], start=True, stop=True)
            nc.vector.tensor_copy(out_sb[0:1, 0, 0:B_SH], op[:])
            nc.gpsimd.trigger_dma(count=None, queue_num=2)

    nc.compile()
    return nc


_NC_CACHE = {}


def _get_nc():
    if 'nc' not in _NC_CACHE:
        _NC_CACHE['nc'] = build()
    return _NC_CACHE['nc']


def kernel(**inputs):
    from concourse.bass_utils import run_bass_kernel_spmd
    nc = _get_nc()
    rep, per_core = prep_inputs(inputs)
    in_maps = [{**rep, **pc} for pc in per_core]
    core_ids = list(range(N_CORES))
    res = run_bass_kernel_spmd(nc, in_maps, core_ids)
    shards = [res.results[i]["out"].reshape(64)[:B_SH] for i in core_ids]
    return np.concatenate(shards).reshape(B, 1).astype(np.float32)


# revision 40
# speedup vs baseline: 1.1912x; 1.1912x over previous
"""Trainium2 Bass kernel for nn_MgSmmSModel_85220741088115 (self-contained).

The reference model is a linear RNN over T=512 steps whose output is a single
scalar per batch element:
  h_t = x_proj_t + h_{t-1} @ W_hc.T;  out = (hT @ W_h.T + ...) @ W_1d.T + b_1d
Because the readout is rank-1, the whole model collapses to a batch-independent
weight functional plus a short dot product over the last J timesteps:
  out[b] = sum_j alpha_j x[b,T-1-j] + s_x x[b,T-1] + beta + c0
  alpha_j = w1d . (W_h W_hc^j w_ic) = u0 . v_j   (u0 = W_h^T w1d, v_j = W_hc^j w_ic)
  beta    = sum_j u0 . y_j                        (y_j = W_hc^j (b_ic+b_hc+b_c))
  c0 = w1d . (b_h + b_g + b_x + rowsum(W_g)) + b_1d;  s_x = w1d . W_x[:,0]
The chain contracts at rho(W_hc) ~ 0.59/step. Weights are fp16 except W_h
(fp8 e4m3 -- u0 tolerates it). J=9 measures 7.122e-3 rel error on hardware,
matching the fp64 host model of the same quantization (2.8x under the
2e-2 gate).

Schedule (per core; all 8 cores run the same program on a batch shard):
  - W_hc^T and W_g^T load via HWDGE (SP queue, whct first); W_h (fp8)
    loads via a *prepared* SWDGE gather (PREPARE_ONLY descriptor gen on
    Pool, fired by trigger_dma) whose completion-to-consumer latency is
    much shorter than the HWDGE path. One prepared load is the sweet spot:
    two or more serialize against each other and regress. The gather idx
    tile holds the wrapped [16,8] iota replicated to all 8 groups of 16
    partitions (HW-verified layout; the consumer uses explicit wait_ge on
    the descriptor DMA semaphore since the prepared-gather dst RAW edge is
    not tile-managed).
  - c0/s_x are computed as a pure bilinear matmul (lhsT chunk k =
    [wx_k | (bh+bg+bx+rowsum)_k] vs rhs w1d_k, plus a b1d column vs ones)
    so only one DVE op sits between W_g^T arriving and the epilogue.
  - The *forward* v/y chain needs only W_hc, so it runs entirely under the
    W_g/W_h loads. Chain steps are column-layout: out[m-chunk][128,2]
    accumulated over 8 contract chunks -> 64 matmuls with out free size 2.
  - Seeds are scaled by 2^10 on device (fp16 subnormal guard); the alpha/beta
    column is scaled back by 2^-10 before the epilogue.
  - beta, s_x and c0 are all folded into the single epilogue matmul: the
    alpha column interleaves (alpha_j, beta_j) in rows 0..23, s_x sits at
    row 32 and c0+b_1d at row 33 (PSUM col base must be 0/32/64/96), and
    xt2 holds x values / 1.0 / x[T-1] / 1.0 in the matching rows.
  - The output store is also a prepared SWDGE scatter-add into a pre-zeroed
    64-f32 DRAM row, fired by a trigger after the final copy -- this skips
    the ~1.3us HWDGE+DGE latency a plain output dma_start would pay.

SPMD over 8 NeuronCores: weight work is replicated (no cross-core collectives:
they cost ~15us flat in the cost model); the batch dim (128) is sharded 16 per
core for the epilogue. Host code does layout/sharding/dtype-cast only.
"""

import numpy as np
import sys
sys.path.insert(0, '/opt/trn_rl_repo')
from concourse import bass, bacc, tile, mybir

F32 = mybir.dt.float32
F16 = mybir.dt.float16
F8 = mybir.dt.float8e4
WH_NP = 'float8_e4m3'   # W_h dtype: u0 tolerates e4m3 (7.2e-3 host rel err)

H = 1024
KT = 8          # 1024 / 128 partition chunks
GT = 4          # 512 / 128 partition chunks (W_g^T rows)
T = 512
B = 128
N_CORES = 8
J = 9           # chain length
C2 = 2 * J      # interleaved (alpha, beta) rows
AB = 34         # ab column length: [0:24] interleaved, 32 = s_x, 33 = c0+b1d
B_SH = B // N_CORES
SC_UP = 1024.0
SC_DN = 1.0 / 1024.0


def col_layout(vec):
    """[1024] -> [128, 8] with element (p, k) = vec[k*128 + p]."""
    return np.ascontiguousarray(vec.reshape(KT, 128).T).astype(np.float32)


def pmaj(mat, nchunks):
    """[nchunks*128, H] -> [128, nchunks*H] with (p, k*H+f) = mat[k*128+p, f]."""
    return np.ascontiguousarray(
        mat.reshape(nchunks, 128, H).transpose(1, 0, 2).reshape(128, nchunks * H))


def _gidx():
    """Gather index tile: wrapped iota (idx[p%16, s] = s*16+p%16), replicated
    to all 8 groups of 16 partitions (one per SDMA engine -- HW-verified)."""
    g = np.zeros((128, KT), np.int16)
    blk = np.arange(128, dtype=np.int16).reshape(KT, 16).T
    for grp in range(8):
        g[grp * 16:(grp + 1) * 16, :] = blk
    return g


def prep_inputs(inputs):
    """Host-side layout/dtype prep only (no arithmetic). -> (replicated, per_core)."""
    x = inputs['x']
    rep = {
        'whct': pmaj(np.ascontiguousarray(inputs['W_hc'].T), KT).astype(np.float16),
        'wh': pmaj(np.asarray(inputs['W_h']), KT).astype(
            np.float16 if WH_NP == 'float16' else __import__('ml_dtypes').float8_e4m3),
        'wgt': pmaj(np.ascontiguousarray(inputs['W_g'].T), GT).astype(np.float16),
        'cols': np.concatenate([
            col_layout(inputs['W_1d'][0]),
            col_layout(inputs['W_ic'][:, 0]),
            col_layout(inputs['W_x'][:, 0]),
            col_layout(inputs['b_ic']),
            col_layout(inputs['b_hc']),
            col_layout(inputs['b_c']),
            col_layout(inputs['b_h']),
            col_layout(inputs['b_g']),
            col_layout(inputs['b_x'])], axis=1),
        'b1d': np.asarray(inputs['b_1d'], np.float32).reshape(1, 1),
    }
    per_core = []
    for i in range(N_CORES):
        xs = x[i * B_SH:(i + 1) * B_SH, T - J:T, 0]      # [B_SH, J]
        xt2 = np.zeros((AB, B_SH), np.float32)
        xt2[0:C2:2, :] = np.ascontiguousarray(xs[:, ::-1].T)  # row 2j = x[., T-1-j]
        xt2[1:C2:2, :] = 1.0                                   # beta rows
        xt2[32, :] = x[i * B_SH:(i + 1) * B_SH, T - 1, 0]      # s_x row
        xt2[33, :] = 1.0                                        # c0 row
        per_core.append({'xt2': xt2})
    return rep, per_core


def build():
    nc = bacc.Bacc("TRN2", target_bir_lowering=False, debug=False,
                   num_devices=N_CORES, num_swdge_queues=3)

    dram = {}
    def din(name, shape, dt=F32):
        dram[name] = nc.dram_tensor(name, list(shape), dt, kind="ExternalInput").ap()
    din('whct', (128, KT * H), F16)
    din('wh', (128, KT * H), F16 if WH_NP == 'float16' else F8)
    din('wgt', (128, GT * H), F16)
    din('cols', (128, 9 * KT))
    din('b1d', (1, 1))
    din('xt2', (AB, B_SH))
    # out row padded to 64 f32 (scatter elem_size granularity is 256B);
    # host slices [:B_SH].
    out_d = nc.dram_tensor("out", [1, 64], F32, kind="ExternalOutput").ap()

    with tile.TileContext(nc) as tc:
        with (
            tc.tile_pool(name="const", bufs=1) as cpool,
            tc.tile_pool(name="psum", bufs=2, space="PSUM") as ppool,
            tc.tile_pool(name="psA", bufs=1, space="PSUM") as ppA,
            tc.tile_pool(name="psB", bufs=1, space="PSUM") as ppB,
            tc.tile_pool(name="psC", bufs=1, space="PSUM") as ppC,
        ):
            # ---- persistent SBUF tiles
            whct_sb = cpool.tile([128, 1, KT * H], F16, tag="whct")
            wh_sb = cpool.tile([128, 1, KT * H],
                               F16 if WH_NP == 'float16' else F8, tag="wh")
            wgt_sb = cpool.tile([128, 1, GT * H], F16, tag="wgt")
            cols_sb = cpool.tile([128, 9 * KT], F32, tag="cols")
            COL_ORDER = ('w1d_c', 'wic_c', 'wx_c', 'bic_c', 'bhc_c', 'bc_c',
                         'bh_c', 'bg_c', 'bx_c')
            colv = {n: cols_sb[:, i * KT:(i + 1) * KT]
                    for i, n in enumerate(COL_ORDER)}
            b1d_sb = cpool.tile([1, 1], F32, tag="b1d")
            xt2_sb = cpool.tile([AB, B_SH], F32, tag="xt2")
            VY = cpool.tile([128, KT, C2], F16, tag="VY")
            w1d16 = cpool.tile([128, KT], F16, tag="w1d16")
            u016 = cpool.tile([128, KT], F16, tag="u016")
            ones16 = cpool.tile([128, 1], F16, tag="ones16")
            onesf = cpool.tile([128, 1], F32, tag="onesf")
            seedf = cpool.tile([128, 2 * KT], F32, tag="seedf")
            bsum = cpool.tile([128, KT], F32, tag="bsum")
            bsum2 = cpool.tile([128, KT], F32, tag="bsum2")
            q2b = cpool.tile([128, KT, 2], F32, tag="q2b")
            b1dcol = cpool.tile([128, 2], F32, tag="b1dcol")
            ab_col = cpool.tile([AB, 1], F32, tag="ab_col")
            out_sb = cpool.tile([128, 1, 64], F32, tag="out_sb")
            oidx = cpool.tile([128, 1], mybir.dt.int16, tag="oidx")
            gidx_sb = cpool.tile([128, KT], mybir.dt.int16, tag="gidx")

            # ---- Transfers. whct first on HWDGE (SP); wgt/wh are prepared
            # SWDGE gathers whose triggers enqueue for the DMA engines in
            # Pool program order -> transfer order whct, (smalls), wgt, wh.
            # Triggered SWDGE completion skips the HWDGE post-transfer
            # latency, shortening the critical wh->u0 hop.
            nc.sync.dma_start(xt2_sb[:], dram['xt2'][:])
            nc.sync.dma_start(wgt_sb[:, 0, :], dram['wgt'][:])
            nc.sync.dma_start(wh_sb[:, 0, :], dram['wh'][:])
            nc.scalar.dma_start(cols_sb[:], dram['cols'][:])
            nc.scalar.dma_start(b1d_sb[:], dram['b1d'][:])

            # on-device gather indices: gidx[p, s] = 16*s + (p & 15)
            # (wrapped iota replicated across all 8 sdma-engine groups)
            gtmp = cpool.tile([128, KT], mybir.dt.int16, tag="gtmp")
            nc.gpsimd.iota(gidx_sb[:], [[16, KT]], base=0, channel_multiplier=0)
            nc.gpsimd.iota(gtmp[:], [[0, KT]], base=0, channel_multiplier=1)
            nc.gpsimd.tensor_scalar(gtmp[:], gtmp[:], 15, None,
                                    mybir.AluOpType.bitwise_and)
            nc.gpsimd.tensor_add(gidx_sb[:], gidx_sb[:], gtmp[:])

            nc.vector.memset(onesf[:], 1.0)
            nc.vector.tensor_copy(ones16[:], onesf[:])
            nc.vector.memset(ab_col[:], 0.0)   # zeros dead rows 24..31
            nc.vector.memset(out_sb[:], 0.0)
            nc.gpsimd.memset(oidx[:], 0)
            # Pre-zero the out row (the scatter-add below accumulates into it)
            nc.scalar.dma_start(out_d[:], out_sb[0:1, 0, :])

            whct_dma_sem = nc.alloc_semaphore("whct_swdge_dma")
            out_dma_sem = nc.alloc_semaphore("out_swdge_dma")
            nc.gpsimd.dma_gather(
                whct_sb[:], dram['whct'][:], gidx_sb[:],
                128, 128, KT * H, prepare_only=True, sem=whct_dma_sem,
                queue_num=1)
            nc.gpsimd.trigger_dma(count=None, queue_num=1)
            # Output-store prep (fired by the trigger at the very end)
            nc.gpsimd.dma_scatter_add(
                out_d[:], out_sb[:], oidx[:], 1, 1, 64,
                prepare_only=True, sem=out_dma_sem, queue_num=2)

            # ---- seeds: v_0 = 2^10 * w_ic, y_0 = 2^10 * (b_ic+b_hc+b_c)
            nc.vector.tensor_scalar_mul(seedf[:, 0:KT], colv['wic_c'], SC_UP)
            nc.vector.tensor_add(bsum[:], colv['bic_c'], colv['bhc_c'])
            nc.vector.tensor_add(bsum[:], bsum[:], colv['bc_c'])
            nc.vector.tensor_scalar_mul(seedf[:, KT:2 * KT], bsum[:], SC_UP)
            nc.vector.tensor_copy(VY[:, :, 0], seedf[:, 0:KT])
            nc.vector.tensor_copy(VY[:, :, 1], seedf[:, KT:2 * KT])
            nc.vector.tensor_copy(w1d16[:], colv['w1d_c'])
            # early constants prep (Activation engine; all inputs are cols)
            nc.vector.tensor_add(bsum2[:], colv['bh_c'], colv['bg_c'])
            nc.vector.tensor_add(bsum2[:], bsum2[:], colv['bx_c'])
            nc.vector.tensor_copy(q2b[:, :, 0], colv['wx_c'])
            nc.vector.memset(b1dcol[:], 0.0)
            nc.vector.tensor_copy(b1dcol[0:1, 1:2], b1d_sb[:])

            # ---- chain: (v,y)_{j+1} = W_hc (v,y)_j, column layout.
            # lhsT tile (k,m) = W_hc^T[k-chunk, m-chunk]; rhs = VY[:, k, 2j:2j+2].
            nc.tensor.wait_ge(whct_dma_sem, 16)
            for j in range(J - 1):
                cp = ppool.tile([128, KT, 2], F32, tag="cp")
                for m in range(KT):
                    for k in range(KT):
                        nc.tensor.matmul(
                            cp[:, m, :],
                            whct_sb[:, 0, k * H + m * 128:k * H + (m + 1) * 128],
                            VY[:, k, 2 * j:2 * j + 2],
                            start=(k == 0), stop=(k == KT - 1))
                nc.vector.tensor_copy(VY[:, :, 2 * (j + 1):2 * (j + 1) + 2], cp[:])

            # ---- rowsum(W_g) via ones: rs[m-chunk] = sum_g W_g[m-chunk, g]
            rs = ppA.tile([128, KT], F32, tag="rs")
            for m in range(KT):
                for g in range(GT):
                    nc.tensor.matmul(
                        rs[:, m:m + 1],
                        wgt_sb[:, 0, g * H + m * 128:g * H + (m + 1) * 128],
                        ones16[:], start=(g == 0), stop=(g == GT - 1))

            # ---- s_x / c0 rows via a bilinear matmul at PSUM base 32:
            # abp[32] = sum wx.w1d, abp[33] = sum (bh+bg+bx+rowsum).w1d + b1d
            # (lhsT chunk k = [wx_k | (bsum2+rs)_k], rhs = w1d_k; the final
            # matmul adds b1d via a [128,2] column against ones).
            abp = ppC.tile([AB, 1], F32, tag="abp")
            nc.vector.tensor_add(q2b[:, :, 1], bsum2[:], rs[:])

            # ---- u0 = W_h^T w1d in column layout
            up = ppA.tile([128, KT], F32, tag="up")
            for m in range(KT):
                for k in range(KT):
                    nc.tensor.matmul(
                        up[:, m:m + 1],
                        wh_sb[:, 0, k * H + m * 128:k * H + (m + 1) * 128],
                        w1d16[:, k:k + 1],
                        start=(k == 0), stop=(k == KT - 1))
            nc.vector.tensor_copy(u016[:], up[:])
            # const matmuls (PE) after u0 so u0 isn't stalled behind them
            for k in range(KT):
                nc.tensor.matmul(abp[32:34, :], q2b[:, k, :],
                                 colv['w1d_c'][:, k:k + 1],
                                 start=(k == 0), stop=False)
            nc.tensor.matmul(abp[32:34, :], b1dcol[:], onesf[:],
                             start=False, stop=True)
            nc.vector.tensor_copy(ab_col[32:34, :], abp[32:34, :])

            # ---- interleaved (alpha_j, beta_j) rows = VY^T u0, scaled 2^-10
            for k in range(KT):
                nc.tensor.matmul(abp[0:C2, :], VY[:, k, :], u016[:, k:k + 1],
                                 start=(k == 0), stop=(k == KT - 1))
            nc.vector.tensor_scalar_mul(ab_col[0:C2, :], abp[0:C2, :], SC_DN)

            # ---- epilogue: out[1, B_SH] = ab_col^T @ xt2 (+ everything folded)
            op = ppB.tile([1, B_SH], F32, tag="op")
            nc.tensor.matmul(op[:], ab_col[:], xt2_sb[:], start=True, stop=True)
            nc.vector.tensor_copy(out_sb[0:1, 0, 0:B_SH], op[:])
            nc.gpsimd.trigger_dma(count=None, queue_num=2)

    nc.compile()
    return nc


_NC_CACHE = {}


def _get_nc():
    if 'nc' not in _NC_CACHE:
        _NC_CACHE['nc'] = build()
    return _NC_CACHE['nc']


def kernel(**inputs):
    from concourse.bass_utils import run_bass_kernel_spmd
    nc = _get_nc()
    rep, per_core = prep_inputs(inputs)
    in_maps = [{**rep, **pc} for pc in per_core]
    core_ids = list(range(N_CORES))
    res = run_bass_kernel_spmd(nc, in_maps, core_ids)
    shards = [res.results[i]["out"].reshape(64)[:B_SH] for i in core_ids]
    return np.concatenate(shards).reshape(B, 1).astype(np.float32)
